# revision 43
# baseline (speedup 1.0000x reference)
"""Multi-head attention Trainium2 kernel (nn_MultiHeadAttention_86423331930281).

Self-contained: builds a Bass/Tile SPMD kernel, data-parallel over batch
(B=8 -> one batch element per NeuronCore), runs on cores 0-7 via
run_bass_kernel_spmd, returns the full [8, 1024, 1024] output.

Per-core algorithm (S=1024, D=1024, H=16, E=64):
  - transpose q/k/v (PE+identity) -> qT/kT/vT [d, s]
  - V-proj:  V[t, he] = vT.T @ Wv  (fp32r), stored as V1 [t, h, 65] with a
    trailing ones column per head (gives softmax denominators for free)
  - per head-pair m: Q/K-proj -> QT/KT [he_pair=128, s] (bf16),
    scoresT = KT_h^T-slices @ QT_h  (K=64 contraction, 2 heads row-packed),
    exp on ScalarE out of PSUM (scale=1/32 folded in) -> P [t, s],
    attendedT[e|sum, s] += [V_h|1].T @ P  accumulated over t in PSUM (fp32r)
  - batched reciprocal of all denominators, broadcast via DRAM round trip,
    normalize attT (bf16), FC: out = attT.T @ WoT + bo (Wo transposed on PE)
"""

import numpy as np
from contextlib import ExitStack

import concourse.bass as bass
import concourse.mybir as mybir
import concourse.tile as tile
from concourse.bass_utils import run_bass_kernel_spmd
from concourse.masks import make_identity

P = 128
S = 1024          # sequence length
DK = 1024         # qkv input dim
H = 16            # heads
E = 64            # per-head dim
HE = H * E        # 1024
OUT = 1024        # output dim
NT = S // P       # 8 s/t tiles
NK = DK // P      # 8 contraction tiles
NM = H // 2       # 8 head pairs
F32 = mybir.dt.float32
F32R = mybir.dt.float32r
BF16 = mybir.dt.bfloat16
AF = mybir.ActivationFunctionType
ALU = mybir.AluOpType
SCALE = 1.0 / 32.0  # 1/sqrt(DK)


def _r(x):
    """bitcast fp32 AP to fp32r for full-rate matmul"""
    return x.bitcast(F32R)


def _legalize_matmul_waits(nc):
    """This walrus build allows only ONE sync-wait command per Matmult.
    Move all but the last wait of any multi-wait matmul onto freshly
    inserted PE nops immediately before it — same engine queue, so the
    blocking semantics are identical."""
    SKIP = ("NoOp", "Br", "Halt", "Sem", "Event")
    k = 0
    for f in nc.m.functions:
        for b in f.blocks:
            out = []
            for inst in b.instructions:
                si = getattr(inst, "sync_info", None)
                tname = type(inst).__name__
                if (not any(s in tname for s in SKIP) and si is not None
                        and si.on_wait and len(si.on_wait) > 1):
                    waits = list(si.on_wait)
                    for w in waits[:-1]:
                        nop = mybir.InstNoOp(
                            name=f"legalize-nop-{k}", ins=[], outs=[])
                        k += 1
                        nop.engine = inst.engine
                        nop.sync_info = mybir.SyncInfo(
                            on_wait=[w], on_update=[])
                        out.append(nop)
                    inst.sync_info = mybir.SyncInfo(
                        on_wait=[waits[-1]], on_update=list(si.on_update))
                out.append(inst)
            b.instructions[:] = out
    return k


def build(legalize=True):
    nc = bass.Bass()
    q_d = nc.dram_tensor("q", (S, DK), F32, kind="ExternalInput")
    k_d = nc.dram_tensor("k", (S, DK), F32, kind="ExternalInput")
    v_d = nc.dram_tensor("v", (S, DK), F32, kind="ExternalInput")
    wq_d = nc.dram_tensor("wq", (H, DK, E), F32, kind="ExternalInput")
    wk_d = nc.dram_tensor("wk", (H, DK, E), F32, kind="ExternalInput")
    wv_d = nc.dram_tensor("wv", (H, DK, E), F32, kind="ExternalInput")
    wo_d = nc.dram_tensor("wo", (OUT, HE), F32, kind="ExternalInput")
    bo_d = nc.dram_tensor("bo", (OUT,), F32, kind="ExternalInput")
    out_d = nc.dram_tensor("out", (S, OUT), F32, kind="ExternalOutput")
    recip_d = nc.dram_tensor("recip_scratch", (H, S), BF16, kind="Internal")

    # [h, d, e] viewed as [di, ko, h, e] so partition = inner contraction dim
    wq_v = wq_d.rearrange("h (ko ki) e -> ki ko h e", ki=P)
    wk_v = wk_d.rearrange("h (ko ki) e -> ki ko h e", ki=P)
    wv_v = wv_d.rearrange("h (ko ki) e -> ki ko h e", ki=P)

    with tile.TileContext(nc) as tc, ExitStack() as ctx:
        const = ctx.enter_context(tc.tile_pool(name="const", bufs=1))
        src = ctx.enter_context(tc.tile_pool(name="src", bufs=3))
        xT = ctx.enter_context(tc.tile_pool(name="xT", bufs=16))
        woTp = ctx.enter_context(tc.tile_pool(name="woTp", bufs=NK))
        v1p = ctx.enter_context(tc.tile_pool(name="v1p", bufs=NT))
        ps = ctx.enter_context(tc.tile_pool(name="ps", bufs=2, space="PSUM"))

        ident = const.tile([P, P], F32, name="ident")
        make_identity(nc, ident)
        ident_bf = const.tile([P, P], BF16, name="ident_bf")
        nc.vector.tensor_copy(ident_bf[:], ident[:])
        bo_bc = const.tile([P, OUT], F32, name="bo_bc")
        nc.sync.dma_start(bo_bc[:], bo_d[None, :].to_broadcast((P, OUT)))
        ones_h = const.tile([P, H], F32, name="ones_h")
        nc.gpsimd.memset(ones_h[:], 1.0)
        sums_all = [const.tile([H // 2, S], F32, name=f"sums_all{i}")
                    for i in range(2)]
        recip_bf = [const.tile([H // 2, S], BF16, name=f"recip_bf{i}")
                    for i in range(2)]

        def transpose_mat(mat_d, name, dt, srcb_scalar=False, tpool=None,
                          pool=None):
            """mat [S, DK] fp32 -> 8 tiles [P, S] of mat.T (tile j = rows j*128..)

            The srcb pass-through both absorbs the multi-queue DMA wait and
            (for bf16) does the downcast; evacuation stays on DVE because the
            BIR verifier only accepts DVE writes as fp32r rounding.
            """
            cast_bf = dt == BF16
            tp = pool if pool is not None else xT
            tag = "xT" if pool is None else "woT"
            tiles = [tp.tile([P, S], dt, name=f"{name}{j}", tag=tag)
                     for j in range(NK)]
            tdt = BF16 if cast_bf else F32
            idt = ident_bf if cast_bf else ident
            dma_engs = [nc.sync, nc.scalar]
            for r in range(NT):
                if cast_bf:
                    # gpsimd DMAs cast in flight: f32 DRAM -> bf16 SBUF
                    stb = src.tile([P, DK], BF16, tag="srcb",
                                   name=f"{name}_srcb{r}")
                    nc.gpsimd.dma_start(stb[:], mat_d[r * P:(r + 1) * P, :])
                else:
                    st = src.tile([P, DK], F32, tag="src", name=f"{name}_src{r}")
                    dma_engs[r % len(dma_engs)].dma_start(
                        st[:], mat_d[r * P:(r + 1) * P, :])
                    stb = st
                for j in range(NK):
                    if tpool is not None:
                        pt_ = tpool.tile([P, P], tdt, tag="tps",
                                         name=f"{name}_ps{r}_{j}")
                    else:
                        pt_ = ps.tile([P, S], tdt, tag="ps",
                                      name=f"{name}_ps{r}_{j}")
                    nc.tensor.transpose(pt_[:, :P], stb[:, j * P:(j + 1) * P], idt[:])
                    dst = tiles[j][:, r * P:(r + 1) * P]
                    if cast_bf and (r + j) % 2 == 1:
                        # bf16 isn't fp32r-rounding-constrained: ACT may evac
                        nc.scalar.copy(dst, pt_[:, :P])
                    else:
                        nc.vector.tensor_copy(dst, pt_[:, :P])
            return tiles

        # first PE instruction: absorb the make_identity (gpsimd) wait into
        # a fresh psum slot (no WAR -> single wait)
        dmy0 = ps.tile([2, P], F32, tag="ps", name="ident_dmy")
        nc.tensor.transpose(dmy0[:2, :P], ident[:, 0:2], ident[:])

        ph1 = ExitStack()
        tps = ph1.enter_context(tc.tile_pool(name="tps", bufs=2, space="PSUM"))
        vT = transpose_mat(v_d, "vT", F32R, srcb_scalar=True, tpool=tps)
        v1_tiles = []
        with tc.tile_pool(name="wv", bufs=NK) as wvp:
            wv_tiles = []
            for j in range(NK):
                raw = src.tile([P, H, E], F32, tag="src", name=f"wvr{j}")
                (nc.sync if j % 2 == 0 else nc.scalar).dma_start(raw[:], wv_v[:, j])
                wt = wvp.tile([P, H, E], F32R, tag="wv", name=f"wv{j}")
                nc.vector.tensor_copy(wt[:], raw[:])
                wv_tiles.append(wt)
            for i in range(NT):
                pst = ps.tile([P, HE], F32, tag="ps", name=f"vproj{i}")
                for nh in range(2):
                    for j in range(NK):
                        wvf = wv_tiles[j][:].rearrange("p h e -> p (h e)")
                        nc.tensor.matmul(
                            pst[:, nh * 512:(nh + 1) * 512],
                            vT[j][:, i * P:(i + 1) * P],
                            wvf[:, nh * 512:(nh + 1) * 512],
                            start=(j == 0), stop=(j == NK - 1))
                v1 = v1p.tile([P, H, E + 1], F32R, tag="v1", name=f"v1_{i}")
                nc.vector.tensor_copy(v1[:, :, E], ones_h[:])
                nc.vector.tensor_copy(
                    v1[:, :, 0:E], pst[:].rearrange("p (h e) -> p h e", e=E))
                v1_tiles.append(v1)

        # ---- phase 1b: transpose q, k
        qT = transpose_mat(q_d, "qT", BF16, tpool=tps)
        kT = transpose_mat(k_d, "kT", BF16, tpool=tps)
        ph1.close()

        # ---- phase 2: per head-pair projections + attention
        wsl = ctx.enter_context(tc.tile_pool(name="wsl", bufs=4))
        qtp = ctx.enter_context(tc.tile_pool(name="qtp", bufs=4))
        ptp = ctx.enter_context(tc.tile_pool(name="ptp", bufs=3))
        attp = ctx.enter_context(tc.tile_pool(name="attp", bufs=NM))
        smallp = ctx.enter_context(tc.tile_pool(name="smallp", bufs=2))
        att_ps = ctx.enter_context(
            tc.tile_pool(name="att_ps", bufs=4, space="PSUM"))

        rbcp = ctx.enter_context(tc.tile_pool(name="rbcp", bufs=2))

        def normalize_batch(ms):
            """reciprocal of denominators for pairs in ms, broadcast, scale"""
            batch = ms[0] // (NM // 2)
            h0 = 2 * ms[0]
            nc.vector.reciprocal(sums_all[batch][:], sums_all[batch][:])
            nc.vector.tensor_copy(recip_bf[batch][:], sums_all[batch][:])
            nc.sync.dma_start(recip_d[h0:h0 + H // 2, :], recip_bf[batch][:])
            for m in ms:
                rbc = rbcp.tile([P, S], BF16, tag="rbc", name=f"rbc{m}")
                for hh in range(2):
                    nc.sync.dma_start(
                        rbc[hh * E:(hh + 1) * E, :],
                        recip_d[2 * m + hh][None, :].to_broadcast((E, S)))
                nc.vector.tensor_tensor(
                    attT_tiles[m][:], attT_tiles[m][:], rbc[:], ALU.mult)

        woT = [woTp.tile([P, S], BF16, name=f"woT{j}", tag="woT")
               for j in range(NK)]

        def wo_row(r):
            stb = src.tile([P, DK], BF16, tag="srcb", name=f"wo_srcb{r}")
            nc.gpsimd.dma_start(stb[:], wo_d[r * P:(r + 1) * P, :])
            for j in range(NK):
                pt_ = ps.tile([P, S], BF16, tag="ps", name=f"wo_ps{r}_{j}")
                nc.tensor.transpose(pt_[:, :P], stb[:, j * P:(j + 1) * P],
                                    ident_bf[:])
                nc.vector.tensor_copy(woT[j][:, r * P:(r + 1) * P], pt_[:, :P])

        attT_tiles = []
        for m in range(NM):
            wqm = wsl.tile([P, NK, 2, E], BF16, tag="wsl", name=f"wq{m}")
            wkm = wsl.tile([P, NK, 2, E], BF16, tag="wsl", name=f"wk{m}")
            wqr = src.tile([P, NK, 2, E], F32, tag="src", name=f"wqr{m}")
            wkr = src.tile([P, NK, 2, E], F32, tag="src", name=f"wkr{m}")
            for hh in range(2):
                nc.sync.dma_start(wqr[:, :, hh, :], wq_v[:, :, 2 * m + hh, :])
                nc.gpsimd.dma_start(wkr[:, :, hh, :], wk_v[:, :, 2 * m + hh, :])
            nc.vector.tensor_copy(wqm[:], wqr[:])
            nc.scalar.copy(wkm[:], wkr[:])

            # QT_m / KT_m: [he_pair=128, s=1024], evacuated as bf16
            qkm = []
            for wm, xtiles, nm in ((wqm, qT, "qtm"), (wkm, kT, "ktm")):
                pst = ps.tile([P, S], F32, tag="ps", name=f"{nm}ps{m}")
                for sh in range(2):
                    for j in range(NK):
                        nc.tensor.matmul(
                            pst[:, sh * 512:(sh + 1) * 512],
                            wm[:, j],
                            xtiles[j][:, sh * 512:(sh + 1) * 512],
                            start=(j == 0), stop=(j == NK - 1))
                t = qtp.tile([P, S], BF16, tag="qt", name=f"{nm}{m}")
                if nm == "qtm":
                    nc.vector.tensor_copy(t[:], pst[:])
                else:
                    nc.scalar.copy(t[:], pst[:])
                qkm.append(t)
            qtm, ktm = qkm

            att_t = {}
            for hh in range(2):
                for sh in range(2):
                    att_t[hh, sh] = att_ps.tile(
                        [E + 1, 512], F32, tag="attps", name=f"att{m}_{hh}_{sh}")
            for j in range(NT):
                for hh in range(2):
                    hs = slice(hh * E, (hh + 1) * E)
                    sc = ps.tile([P, S], F32, tag="ps", name=f"sc{m}_{j}_{hh}")
                    for sh in range(2):
                        nc.tensor.matmul(
                            sc[:, sh * 512:(sh + 1) * 512],
                            ktm[hs, j * P:(j + 1) * P],
                            qtm[hs, sh * 512:(sh + 1) * 512],
                            start=True, stop=True)
                    ptile = ptp.tile([P, S], F32R, tag="pt", name=f"p{m}_{j}_{hh}")
                    nc.scalar.activation(ptile[:], sc[:], AF.Exp, scale=SCALE)
                    for sh in range(2):
                        nc.tensor.matmul(
                            att_t[hh, sh][:],
                            v1_tiles[j][:, 2 * m + hh, :],
                            ptile[:, sh * 512:(sh + 1) * 512],
                            start=(j == 0), stop=(j == NT - 1))

            # evacuate attendedT + denominators (unnormalized, bf16)
            attm = attp.tile([P, S], BF16, tag="attT", name=f"attT{m}")
            attT_tiles.append(attm)
            for hh in range(2):
                for sh in range(2):
                    apt = att_t[hh, sh]
                    stg = smallp.tile([E + 1, 512], F32, tag="stage",
                                      name=f"stg{m}_{hh}_{sh}")
                    nc.vector.tensor_copy(stg[E:E + 1, :], apt[E:E + 1, :])
                    row = (2 * m + hh) % (H // 2)
                    nc.sync.dma_start(
                        sums_all[m // (NM // 2)][row:row + 1,
                                                 sh * 512:(sh + 1) * 512],
                        stg[E:E + 1, :])
                    nc.vector.tensor_copy(
                        attm[hh * E:(hh + 1) * E, sh * 512:(sh + 1) * 512],
                        apt[0:E, :])
            if m == NM // 2 - 1:
                normalize_batch(list(range(NM // 2)))

        # ---- phase 3: transpose Wo, normalize second half, FC
        for r in range(NT):
            wo_row(r)
        normalize_batch(list(range(NM // 2, NM)))

        outp = ctx.enter_context(tc.tile_pool(name="outp", bufs=2))
        for st in range(NT):
            for oh in range(2):
                pso = att_ps.tile([P, 512], F32, tag="attps",
                                  name=f"fc{st}_{oh}")
                for m in range(NM):
                    nc.tensor.matmul(
                        pso[:],
                        attT_tiles[m][:, st * P:(st + 1) * P],
                        woT[m][:, oh * 512:(oh + 1) * 512],
                        start=(m == 0), stop=(m == NM - 1))
                ot = outp.tile([P, 512], F32, tag="out", name=f"out{st}_{oh}")
                nc.vector.tensor_tensor(
                    ot[:], pso[:], bo_bc[:, oh * 512:(oh + 1) * 512],
                    ALU.add)
                nc.sync.dma_start(
                    out_d[st * P:(st + 1) * P, oh * 512:(oh + 1) * 512], ot[:])
    if legalize:
        _legalize_matmul_waits(nc)
    return nc


_NC_CACHE = {}


def _get_nc():
    if "nc" not in _NC_CACHE:
        _NC_CACHE["nc"] = build()
    return _NC_CACHE["nc"]


def kernel(query, key, value, Wq, Wk, Wv, Wo, bo, **run_kwargs):
    query = np.asarray(query, dtype=np.float32)
    key = np.asarray(key, dtype=np.float32)
    value = np.asarray(value, dtype=np.float32)
    Wq = np.ascontiguousarray(np.asarray(Wq, dtype=np.float32))
    Wk = np.ascontiguousarray(np.asarray(Wk, dtype=np.float32))
    Wv = np.ascontiguousarray(np.asarray(Wv, dtype=np.float32))
    Wo = np.ascontiguousarray(np.asarray(Wo, dtype=np.float32))
    bo = np.ascontiguousarray(np.asarray(bo, dtype=np.float32))
    B = query.shape[0]
    assert B == 8, f"expected batch 8, got {B}"

    nc = _get_nc()
    in_maps = []
    for b in range(B):
        in_maps.append({
            "q": np.ascontiguousarray(query[b]),
            "k": np.ascontiguousarray(key[b]),
            "v": np.ascontiguousarray(value[b]),
            "wq": Wq, "wk": Wk, "wv": Wv, "wo": Wo, "bo": bo,
        })
    res = run_bass_kernel_spmd(nc, in_maps, core_ids=list(range(B)),
                               **run_kwargs)
    out = np.stack([r["out"] for r in res.results], axis=0)
    if run_kwargs.get("trace"):
        _NC_CACHE["last_result"] = res
    return out
